# revision 13
# baseline (speedup 1.0000x reference)
"""ExpLeak (leaky integrator) Trainium2 kernel.

Computes, over a [B=16, T=1024, N=4096] f32 tensor:
    y[b, t, n] = alpha * y[b, t-1, n] + x[b, t, n],   alpha = exp(-1/tau)

Strategy
--------
Pure data parallel: 8 NeuronCores x 2 batches each.  Per core the host
lays x out time-major as [T, W=B_PER*N] fp16, and the time recurrence
is evaluated as a blocked lower-triangular matmul over chunks of C=127
steps:

    y_chunk = L @ x_chunk + alphas (x) carry
    L[t, s]    = alpha^(t-s)  for s <= t, else 0      (127 x 127)
    alphas[t]  = alpha^(t+1)                          (1 x 127)
    carry      = y[last row of previous chunk]

Using C=127 (not 128) is the key trick: the L^T weight matrix gets an
extra column 0 computing the NEXT chunk's carry (sum alpha^(126-s) x_s
+ alpha^127 carry = y[last]), so the [128, 512] PSUM tile holds the
carry in row 0 and the 127 y rows in rows 1..127.  The one PSUM->SBUF
copy lands the carry at SBUF partition 0, where the next chunk's K=1
carry matmul can legally read it in place (matmul rhs base partition
must be 0/32/64) -- the carry never round-trips through HBM or a
partition-moving DMA.  Each of the 16 feature slices forms its own
independent carry chain (copy -> next-chunk matmul), so chain latency
hides behind the other slices' work.  Stores read rows 1..128 of the
output tile.  T=1024 = 8*127 + 8: the last 8 steps are a runt chunk
using a partition/column slice of the same weights (its carry output
is junk, and unused).

Both terms accumulate in one fp32 PSUM bank per [127,512] slice; the
PSUM->SBUF copies alternate between the DVE and ACT engines.  Loads
ride the SP HWDGE ring, stores the ACT ring.

I/O precision: the kernel is memory-bound, so x and y ride HBM as
float16 (host casts round-to-nearest; fp16 weights).  End-to-end rms
relative error ~3e-4, inside the 1e-3 target, at half the f32 traffic.
"""

import os
import sys

import numpy as np


def _ensure_concourse():
    try:
        import concourse.bass  # noqa: F401
        return
    except ImportError:
        pass
    for p in ("/opt/trn_rl_repo", "/root/.axon_site/_ro/trn_rl_repo"):
        if os.path.isdir(p) and p not in sys.path:
            sys.path.insert(0, p)
    import concourse.bass  # noqa: F401


B, T, N = 16, 1024, 4096
N_CORES = 8
B_PER = B // N_CORES      # batches per core
W = B_PER * N             # time-major row width (8192)
C = 127                   # chunk: 127 time steps + the carry partition
NFULL = T // C            # 8 full chunks
RT = T - NFULL * C        # runt chunk length (8)
FT = 512                  # feature tile (one fp32 PSUM bank)
NFT = W // FT

_PROGRAM_CACHE = {}


def build_program(repeats=None, variant="full", io="fp16"):
    """Trace + compile the per-core Bass/Tile program.  alpha enters
    only through the lt/av input tensors, so one program serves any tau.

    repeats: if set, wrap the whole body in a tc.For_i loop that redoes
    the identical (idempotent) computation `repeats` times — used by
    test.py to measure the steady-state kernel time as a slope,
    independent of the per-launch dispatch overhead."""
    _ensure_concourse()
    import contextlib

    import concourse.bacc as bacc
    import concourse.mybir as mybir
    from concourse import tile

    assert io == "fp16"
    DIO = mybir.dt.float16
    DT = mybir.dt.float32

    nc = bacc.Bacc("TRN2", target_bir_lowering=False, debug=False,
                   num_devices=N_CORES)
    x = nc.declare_dram_parameter("x", [T, W], DIO, isOutput=False)
    lt = nc.declare_dram_parameter("lt", [C, C + 1], DIO, isOutput=False)
    av = nc.declare_dram_parameter("av", [1, C + 1], DIO, isOutput=False)
    y = nc.declare_dram_parameter("y", [T, W], DIO, isOutput=True)

    with tile.TileContext(nc) as tc:
        with (
            tc.tile_pool(name="w", bufs=1) as wpool,
            tc.tile_pool(name="xp", bufs=3) as xpool,
            tc.tile_pool(name="op", bufs=3) as opool,
            tc.tile_pool(name="ps", bufs=8, space="PSUM") as pspool,
        ):
            ltt = wpool.tile([C, C + 1], DIO, tag="lt")
            nc.sync.dma_start(ltt[:], lt[:])
            avt = wpool.tile([1, C + 1], DIO, tag="av")
            nc.sync.dma_start(avt[:], av[:])

            rep = (tc.For_i(0, repeats, 1, staggered_reset=True,
                            hint_engines=(mybir.EngineType.PE,))
                   if repeats else contextlib.nullcontext())
            with rep:
                _emit_body(nc, tc, x, y, xpool, opool, pspool,
                           ltt, avt, DIO, DT, mybir, variant)

    nc.compile()
    return nc


def _emit_body(nc, tc, x, y, xpool, opool, pspool, ltt, avt, DIO, DT,
               mybir, variant="full"):
    prev_ot = None
    for k in range(NFULL + 1):
        t0 = k * C
        rows = C if k < NFULL else RT
        xt = xpool.tile([rows, W], DIO, tag="xt")
        nc.sync.dma_start(xt[:], x[t0:t0 + rows, :])
        if variant == "dma":
            # measurement-only: pure load->store roundtrip
            nc.scalar.dma_start(y[t0:t0 + rows, :], xt[:])
            continue
        # row 0 = next chunk's carry, rows 1..rows = this chunk's y
        ot = opool.tile([rows + 1, W], DIO, tag="ot")
        for j in range(NFT):
            fsl = slice(j * FT, (j + 1) * FT)
            ps = pspool.tile([rows + 1, FT], DT, tag="ps")
            nc.tensor.matmul(
                ps[:],
                ltt[0:rows, 0:rows + 1],
                xt[:, fsl],
                start=True,
                stop=(k == 0),
            )
            if k > 0:
                # carry: K=1 matmul reading the previous chunk's carry
                # row in place at partition 0 of its SBUF output tile
                nc.tensor.matmul(
                    ps[:],
                    avt[0:1, 0:rows + 1],
                    prev_ot[0:1, fsl],
                    start=False,
                    stop=True,
                )
            # PSUM -> SBUF (cast to fp16), alternating engines
            if j % 2 == 0:
                nc.vector.tensor_copy(ot[:, fsl], ps[:])
            else:
                nc.scalar.copy(ot[:, fsl], ps[:])
        nc.scalar.dma_start(y[t0:t0 + rows, :], ot[1:rows + 1, :])
        prev_ot = ot


def _get_program():
    nc = _PROGRAM_CACHE.get("nc")
    if nc is None:
        nc = build_program()
        _PROGRAM_CACHE["nc"] = nc
    return nc


def make_weights(alpha: float):
    """lt [C, C+1]: column 0 is the carry generator alpha^(C-1-s);
    column m (1..C) is alpha^((m-1)-s) for s <= m-1 (the L^T block).
    av [1, C+1]: av[0] = alpha^C (carry feedback), av[m] = alpha^m.
    Both fp16."""
    powers = np.power(np.float64(alpha), np.arange(C + 1)).astype(np.float32)
    lt = np.zeros((C, C + 1), dtype=np.float32)
    lt[:, 0] = powers[C - 1 - np.arange(C)]
    s_idx, m_idx = np.meshgrid(np.arange(C), np.arange(1, C + 1),
                               indexing="ij")
    mask = s_idx <= m_idx - 1
    lt[:, 1:][mask] = powers[(m_idx - 1 - s_idx)[mask]]
    av = powers[np.arange(C + 1)].reshape(1, C + 1).copy()
    av[0, 0] = powers[C]
    return lt.astype(np.float16), av.astype(np.float16)


def prepare_in_maps(input_current: np.ndarray, tau_mem: np.ndarray,
                    io="fp16"):
    """Shard + cast + lay out time-major into per-core dicts."""
    tau = np.float32(np.asarray(tau_mem).reshape(-1)[0])
    alpha = float(np.exp(np.float32(-1.0) / tau))
    lt, av = make_weights(alpha)
    x = np.asarray(input_current)
    maps = []
    for c in range(N_CORES):
        xc = x[c * B_PER:(c + 1) * B_PER]        # [B_PER, T, N]
        xtm = np.ascontiguousarray(
            xc.transpose(1, 0, 2), dtype=np.float16).reshape(T, W)
        maps.append({"x": xtm, "lt": lt, "av": av})
    return maps


def kernel(input_current: np.ndarray, tau_mem: np.ndarray) -> np.ndarray:
    _ensure_concourse()
    from concourse.bass_utils import run_bass_kernel_spmd

    nc = _get_program()
    in_maps = prepare_in_maps(input_current, tau_mem)
    res = run_bass_kernel_spmd(nc, in_maps, list(range(N_CORES)))
    parts = []
    for c in range(N_CORES):
        ytm = res.results[c]["y"].reshape(T, B_PER, N)
        parts.append(
            np.ascontiguousarray(ytm.transpose(1, 0, 2), dtype=np.float32))
    return np.concatenate(parts, axis=0)


# revision 17
# speedup vs baseline: 1.5726x; 1.5726x over previous
"""ExpLeak (leaky integrator) Trainium2 kernel.

Computes, over a [B=16, T=1024, N=4096] f32 tensor:
    y[b, t, n] = alpha * y[b, t-1, n] + x[b, t, n],   alpha = exp(-1/tau)

Strategy
--------
Pure data parallel: 8 NeuronCores x 2 batches each.  Per core the host
lays x out time-major as [T, W=B_PER*N] fp16, and the time recurrence
is evaluated as a blocked lower-triangular matmul over chunks of C=127
steps:

    y_chunk = L @ x_chunk + alphas (x) carry
    L[t, s]    = alpha^(t-s)  for s <= t, else 0      (127 x 127)
    alphas[t]  = alpha^(t+1)                          (1 x 127)
    carry      = y[last row of previous chunk]

Using C=127 (not 128) is the key trick: the L^T weight matrix gets an
extra column 0 computing the NEXT chunk's carry (sum alpha^(126-s) x_s
+ alpha^127 carry = y[last]), so the [128, 512] PSUM tile holds the
carry in row 0 and the 127 y rows in rows 1..127.  The one PSUM->SBUF
copy lands the carry at SBUF partition 0, where the next chunk's K=1
carry matmul can legally read it in place (matmul rhs base partition
must be 0/32/64) -- the carry never round-trips through HBM or a
partition-moving DMA.  Each of the 16 feature slices forms its own
independent carry chain (copy -> next-chunk matmul), so chain latency
hides behind the other slices' work.  Stores read rows 1..128 of the
output tile.  T=1024 = 8*127 + 8: the last 8 steps are a runt chunk
using a partition/column slice of the same weights (its carry output
is junk, and unused).

Both terms accumulate in one fp32 PSUM bank per [127,512] slice; the
PSUM->SBUF copies alternate between the DVE and ACT engines.  Loads
ride the SP HWDGE ring, stores the ACT ring.

I/O precision: the kernel is memory-bound, so x and y ride HBM as
float16 (host casts round-to-nearest; fp16 weights).  End-to-end rms
relative error ~3e-4, inside the 1e-3 target, at half the f32 traffic.
"""

import os
import sys

import numpy as np


def _ensure_concourse():
    try:
        import concourse.bass  # noqa: F401
        return
    except ImportError:
        pass
    for p in ("/opt/trn_rl_repo", "/root/.axon_site/_ro/trn_rl_repo"):
        if os.path.isdir(p) and p not in sys.path:
            sys.path.insert(0, p)
    import concourse.bass  # noqa: F401


B, T, N = 16, 1024, 4096
N_CORES = 8
B_PER = B // N_CORES      # batches per core
W = B_PER * N             # time-major row width (8192)
C = 127                   # chunk: 127 time steps + the carry partition
NFULL = T // C            # 8 full chunks
RT = T - NFULL * C        # runt chunk length (8)
FT = 512                  # feature tile (one fp32 PSUM bank)
NFT = W // FT

_PROGRAM_CACHE = {}


def build_program(repeats=None, variant="full", io="fp16"):
    """Trace + compile the per-core Bass/Tile program.  alpha enters
    only through the lt/av input tensors, so one program serves any tau.

    repeats: if set, wrap the whole body in a tc.For_i loop that redoes
    the identical (idempotent) computation `repeats` times — used by
    test.py to measure the steady-state kernel time as a slope,
    independent of the per-launch dispatch overhead."""
    _ensure_concourse()
    import contextlib

    import concourse.bacc as bacc
    import concourse.mybir as mybir
    from concourse import tile

    assert io == "fp16"
    DIO = mybir.dt.float16
    DT = mybir.dt.float32

    nc = bacc.Bacc("TRN2", target_bir_lowering=False, debug=False,
                   num_devices=N_CORES)
    x = nc.declare_dram_parameter("x", [T, W], DIO, isOutput=False)
    # lt has a 128th row of ZEROS: the main matmul contracts over K=128
    # (127 x rows + one zero-weighted overlap row) because Fast Weight
    # Load requires NumWeights==128 — at K=127 the PE loads the
    # stationary operand one element per cycle (~6.8us per matmul, 4x
    # overall) instead of pipelined behind the previous matmul.
    lt = nc.declare_dram_parameter("lt", [C + 1, C + 1], DIO, isOutput=False)
    av = nc.declare_dram_parameter("av", [1, C + 1], DIO, isOutput=False)
    y = nc.declare_dram_parameter("y", [T, W], DIO, isOutput=True)

    with tile.TileContext(nc) as tc:
        with (
            tc.tile_pool(name="w", bufs=1) as wpool,
            tc.tile_pool(name="xp", bufs=3) as xpool,
            tc.tile_pool(name="op", bufs=3) as opool,
            tc.tile_pool(name="ps", bufs=8, space="PSUM") as pspool,
        ):
            ltt = wpool.tile([C + 1, C + 1], DIO, tag="lt")
            nc.sync.dma_start(ltt[:], lt[:])
            avt = wpool.tile([1, C + 1], DIO, tag="av")
            nc.sync.dma_start(avt[:], av[:])

            rep = (tc.For_i(0, repeats, 1, staggered_reset=True,
                            hint_engines=(mybir.EngineType.PE,))
                   if repeats else contextlib.nullcontext())
            with rep:
                _emit_body(nc, tc, x, y, xpool, opool, pspool,
                           ltt, avt, DIO, DT, mybir, variant)

    nc.compile()
    return nc


def _emit_body(nc, tc, x, y, xpool, opool, pspool, ltt, avt, DIO, DT,
               mybir, variant="full"):
    prev_ot = None
    for k in range(NFULL + 1):
        t0 = k * C
        rows = C if k < NFULL else RT
        # full chunks load C+1=128 rows (one overlap row, zero weight)
        # so the contraction is K=128 and FWL stays enabled
        krows = rows + 1 if k < NFULL else rows
        xt = xpool.tile([krows, W], DIO, tag="xt")
        nc.sync.dma_start(xt[:], x[t0:t0 + krows, :])
        if variant == "dma":
            # measurement-only: pure load->store roundtrip
            nc.scalar.dma_start(y[t0:t0 + krows, :], xt[:])
            continue
        # row 0 = next chunk's carry, rows 1..rows = this chunk's y
        ot = opool.tile([rows + 1, W], DIO, tag="ot")
        for j in range(NFT):
            fsl = slice(j * FT, (j + 1) * FT)
            ps = pspool.tile([rows + 1, FT], DT, tag="ps")
            nc.tensor.matmul(
                ps[:],
                ltt[0:krows, 0:rows + 1],
                xt[:, fsl],
                start=True,
                stop=(k == 0),
            )
            if k > 0:
                # carry: K=1 matmul reading the previous chunk's carry
                # row in place at partition 0 of its SBUF output tile
                nc.tensor.matmul(
                    ps[:],
                    avt[0:1, 0:rows + 1],
                    prev_ot[0:1, fsl],
                    start=False,
                    stop=True,
                )
            # PSUM -> SBUF (cast to fp16), alternating engines
            if j % 2 == 0:
                nc.vector.tensor_copy(ot[:, fsl], ps[:])
            else:
                nc.scalar.copy(ot[:, fsl], ps[:])
        nc.scalar.dma_start(y[t0:t0 + rows, :], ot[1:rows + 1, :])
        prev_ot = ot


def _get_program():
    nc = _PROGRAM_CACHE.get("nc")
    if nc is None:
        nc = build_program()
        _PROGRAM_CACHE["nc"] = nc
    return nc


def make_weights(alpha: float):
    """lt [C+1, C+1]: column 0 is the carry generator alpha^(C-1-s);
    column m (1..C) is alpha^((m-1)-s) for s <= m-1 (the L^T block);
    row C is all zeros (the K=128 / FWL padding row).
    av [1, C+1]: av[0] = alpha^C (carry feedback), av[m] = alpha^m.
    Both fp16."""
    powers = np.power(np.float64(alpha), np.arange(C + 1)).astype(np.float32)
    lt = np.zeros((C + 1, C + 1), dtype=np.float32)
    lt[0:C, 0] = powers[C - 1 - np.arange(C)]
    s_idx, m_idx = np.meshgrid(np.arange(C), np.arange(1, C + 1),
                               indexing="ij")
    mask = s_idx <= m_idx - 1
    lt[0:C, 1:][mask] = powers[(m_idx - 1 - s_idx)[mask]]
    av = powers[np.arange(C + 1)].reshape(1, C + 1).copy()
    av[0, 0] = powers[C]
    return lt.astype(np.float16), av.astype(np.float16)


def prepare_in_maps(input_current: np.ndarray, tau_mem: np.ndarray,
                    io="fp16"):
    """Shard + cast + lay out time-major into per-core dicts."""
    tau = np.float32(np.asarray(tau_mem).reshape(-1)[0])
    alpha = float(np.exp(np.float32(-1.0) / tau))
    lt, av = make_weights(alpha)
    x = np.asarray(input_current)
    maps = []
    for c in range(N_CORES):
        xc = x[c * B_PER:(c + 1) * B_PER]        # [B_PER, T, N]
        xtm = np.ascontiguousarray(
            xc.transpose(1, 0, 2), dtype=np.float16).reshape(T, W)
        maps.append({"x": xtm, "lt": lt, "av": av})
    return maps


def kernel(input_current: np.ndarray, tau_mem: np.ndarray) -> np.ndarray:
    _ensure_concourse()
    from concourse.bass_utils import run_bass_kernel_spmd

    nc = _get_program()
    in_maps = prepare_in_maps(input_current, tau_mem)
    res = run_bass_kernel_spmd(nc, in_maps, list(range(N_CORES)))
    parts = []
    for c in range(N_CORES):
        ytm = res.results[c]["y"].reshape(T, B_PER, N)
        parts.append(
            np.ascontiguousarray(ytm.transpose(1, 0, 2), dtype=np.float32))
    return np.concatenate(parts, axis=0)


# revision 18
# speedup vs baseline: 5.9074x; 3.7564x over previous
"""ExpLeak (leaky integrator) Trainium2 kernel.

Computes, over a [B=16, T=1024, N=4096] f32 tensor:
    y[b, t, n] = alpha * y[b, t-1, n] + x[b, t, n],   alpha = exp(-1/tau)

Strategy
--------
Pure data parallel over batch: 8 NeuronCores x 2 batches each.

Per core, the time recurrence is evaluated as a blocked lower-triangular
matmul.  For a time chunk of C=128 steps,

    y_chunk = L @ x_chunk + alphas (x) carry          (outer product)
    L[t, s]    = alpha^(t-s)  for s <= t, else 0
    alphas[t]  = alpha^(t+1)
    carry[n]   = y[last row of previous chunk, n]

Both terms are PE matmuls accumulating into the same PSUM bank:
  - main:  lhsT = L^T  [128,128], rhs = x tile slice [128, 512]
  - carry: lhsT = alphas [1,128], rhs = carry row    [1,   512]  (K=1)
The carry row for the next chunk is the out row 127, moved to partition
0 of an SBUF tile with a small SWDGE DMA.

I/O precision: the kernel is memory-bound (HBM roofline), so x and y
ride HBM as float16 (host casts f32->fp16 with round-to-nearest).  The
PE multiplies fp16 at full rate and accumulates in fp32 PSUM; the L
weights are fp16 (e5m10, 4.9e-4 ulp), so the end-to-end rms relative
error is ~4e-4 -- well inside the 1e-3 target -- while HBM traffic
halves vs f32 (32 MiB -> 16 MiB per core each way).
"""

import os
import sys

import numpy as np


def _ensure_concourse():
    try:
        import concourse.bass  # noqa: F401
        return
    except ImportError:
        pass
    for p in ("/opt/trn_rl_repo", "/root/.axon_site/_ro/trn_rl_repo"):
        if os.path.isdir(p) and p not in sys.path:
            sys.path.insert(0, p)
    import concourse.bass  # noqa: F401


B, T, N = 16, 1024, 4096
N_CORES = 8
B_PER = B // N_CORES  # batches per core
C = 128               # time chunk (PE contraction dim)
NCHUNK = T // C
FT = 512              # feature tile (max f32 PSUM bank free dim)
NFT = N // FT

_PROGRAM_CACHE = {}


def build_program(repeats=None, variant="full", io="fp16"):
    """Trace + compile the per-core Bass/Tile program. alpha enters only
    through the lt/av input tensors, so one program serves any tau.

    repeats: if set, wrap the whole body in a tc.For_i loop that redoes
    the identical (idempotent) computation `repeats` times — used by
    test.py to measure the steady-state kernel time as a slope,
    independent of the per-launch dispatch overhead.

    io: "fp16" (default) or "fp32" — dtype of x/y in HBM and of the PE
    operands.  fp32 uses fp32r matmuls with Dekker-split weights."""
    _ensure_concourse()
    import contextlib

    import concourse.bacc as bacc
    import concourse.mybir as mybir
    from concourse import tile

    DT = mybir.dt.float32
    if io == "fp16":
        DIO = mybir.dt.float16   # HBM dtype of x / y
        DPE = mybir.dt.float16   # PE operand dtype
    else:
        DIO = mybir.dt.float32
        DPE = mybir.dt.float32r

    nc = bacc.Bacc("TRN2", target_bir_lowering=False, debug=False,
                   num_devices=N_CORES)
    x = nc.declare_dram_parameter("x", [B_PER, T, N], DIO, isOutput=False)
    lt = nc.declare_dram_parameter("lt", [C, C], DIO, isOutput=False)
    ltl = None
    if io == "fp32":
        ltl = nc.declare_dram_parameter("ltl", [C, C], DIO, isOutput=False)
    av = nc.declare_dram_parameter("av", [1, C], DIO, isOutput=False)
    y = nc.declare_dram_parameter("y", [B_PER, T, N], DIO, isOutput=True)

    def as_pe(ap):
        return ap.bitcast(DPE) if io == "fp32" else ap

    with tile.TileContext(nc) as tc:
        with (
            tc.tile_pool(name="w", bufs=1) as wpool,
            tc.tile_pool(name="xp", bufs=6) as xpool,
            tc.tile_pool(name="op", bufs=3) as opool,
            tc.tile_pool(name="cp", bufs=2) as cpool,
            tc.tile_pool(name="ps", bufs=8, space="PSUM") as pspool,
        ):
            # fp32 path: the PE reads the top 20 bits (e8m11) of fp32r;
            # weights are pre-rounded on host and L^T is Dekker-split
            # into hi+lo so the main-matmul weights are exact to fp32.
            # fp16 path: weights are plain fp16, single matmul.
            ltt = wpool.tile([C, C], DPE, tag="lt")
            nc.sync.dma_start(ltt[:], as_pe(lt[:]))
            ltlt = None
            if io == "fp32":
                ltlt = wpool.tile([C, C], DPE, tag="ltl")
                nc.sync.dma_start(ltlt[:], as_pe(ltl[:]))
            avt = wpool.tile([1, C], DPE, tag="av")
            nc.sync.dma_start(avt[:], as_pe(av[:]))

            rep = (tc.For_i(0, repeats, 1, staggered_reset=True,
                            hint_engines=(mybir.EngineType.PE,))
                   if repeats else contextlib.nullcontext())
            with rep:
                _emit_body(nc, tc, x, y, xpool, opool, cpool, pspool,
                           ltt, ltlt, avt, DT, DPE, as_pe, mybir, variant)

    nc.compile()
    return nc


def _emit_body(nc, tc, x, y, xpool, opool, cpool, pspool,
               ltt, ltlt, avt, DT, DPE, as_pe, mybir, variant="full"):
    carry = {}
    for k in range(NCHUNK):
        trange = slice(k * C, (k + 1) * C)
        for b in range(B_PER):
            xt = xpool.tile([C, N], DPE, tag="xt")
            # two halves: earlier half-completion lets dependent
            # matmuls start sooner (~1% in A/B vs one whole-tile DMA)
            nc.sync.dma_start(xt[:, 0:N // 2], as_pe(x[b, trange, 0:N // 2]))
            nc.sync.dma_start(xt[:, N // 2:N], as_pe(x[b, trange, N // 2:N]))
            if variant == "dma":
                # measurement-only: pure load->store roundtrip
                nc.scalar.dma_start(y[b, trange, 0:N // 2],
                                    as_pe(xt[:, 0:N // 2]))
                nc.scalar.dma_start(y[b, trange, N // 2:N],
                                    as_pe(xt[:, N // 2:N]))
                continue
            ot = opool.tile([C, N], DPE, tag="ot")
            newcarry = cpool.tile([1, N], DPE, tag="carry")
            for j in range(NFT):
                fsl = slice(j * FT, (j + 1) * FT)
                ps = pspool.tile([C, FT], DT, tag="ps")
                nc.tensor.matmul(
                    ps[:],
                    ltt[:],
                    xt[:, fsl],
                    start=True,
                    stop=(k == 0 and ltlt is None),
                )
                if ltlt is not None:
                    nc.tensor.matmul(
                        ps[:],
                        ltlt[:],
                        xt[:, fsl],
                        start=False,
                        stop=(k == 0),
                    )
                if k > 0:
                    nc.tensor.matmul(
                        ps[:],
                        avt[:],
                        carry[b][0:1, fsl],
                        start=False,
                        stop=True,
                    )
                nc.vector.tensor_copy(ot[:, fsl], ps[:])
            # next chunk's carry: out row 127 -> partition 0 (SWDGE
            # keeps this dependent little DMA out of the HWDGE FIFOs).
            nc.gpsimd.dma_start(newcarry[0:1, :], ot[C - 1:C, :])
            # stores ride the ACT HWDGE ring so the SP ring only
            # carries loads and streams ahead (measured best).
            nc.scalar.dma_start(y[b, trange, 0:N // 2],
                                as_pe(ot[:, 0:N // 2]))
            nc.scalar.dma_start(y[b, trange, N // 2:N],
                                as_pe(ot[:, N // 2:N]))
            carry[b] = newcarry


def _get_program():
    nc = _PROGRAM_CACHE.get("nc")
    if nc is None:
        nc = build_program()
        _PROGRAM_CACHE["nc"] = nc
    return nc


def _round_fp32r(a: np.ndarray) -> np.ndarray:
    """Round fp32 to the PE's fp32r grid (e8m11: low 12 mantissa bits
    zero), round-to-nearest-even."""
    bits = a.astype(np.float32).view(np.uint32)
    keep = np.uint32(0xFFFFF000)
    low = bits & np.uint32(0xFFF)
    lsb = (bits >> np.uint32(12)) & np.uint32(1)
    round_up = (low > 0x800) | ((low == 0x800) & (lsb == 1))
    out = (bits & keep) + np.where(round_up, np.uint32(0x1000), np.uint32(0))
    return out.view(np.float32)


def make_weights(alpha: float, io="fp16"):
    """Host-side constant tensors.
    fp16: lt = L^T and av[0,t] = alpha^(t+1), both rounded to fp16.
    fp32: lt/ltl = hi/lo Dekker split of L^T on the fp32r grid."""
    powers = np.power(np.float64(alpha), np.arange(C + 1))
    lt = np.zeros((C, C), dtype=np.float32)
    s_idx, t_idx = np.meshgrid(np.arange(C), np.arange(C), indexing="ij")
    mask = s_idx <= t_idx
    lt[mask] = powers[(t_idx - s_idx)[mask]].astype(np.float32)
    av = powers[1:].astype(np.float32).reshape(1, C)
    if io == "fp16":
        return lt.astype(np.float16), None, av.astype(np.float16)
    lt_hi = _round_fp32r(lt)
    lt_lo = _round_fp32r((lt - lt_hi).astype(np.float32))
    return lt_hi, lt_lo, _round_fp32r(av)


def prepare_in_maps(input_current: np.ndarray, tau_mem: np.ndarray,
                    io="fp16"):
    """Shard + cast the full inputs into per-core parameter dicts."""
    tau = np.float32(np.asarray(tau_mem).reshape(-1)[0])
    alpha = float(np.exp(np.float32(-1.0) / tau))
    lt, ltl, av = make_weights(alpha, io=io)
    x = np.ascontiguousarray(input_current, dtype=np.float32)
    if io == "fp16":
        x = x.astype(np.float16)
    else:
        # round-to-nearest onto the fp32r grid (instead of the PE's
        # truncation of the low 12 bits: halves the input error)
        x = _round_fp32r(x)
    maps = []
    for c in range(N_CORES):
        m = {"x": x[c * B_PER:(c + 1) * B_PER], "lt": lt, "av": av}
        if ltl is not None:
            m["ltl"] = ltl
        maps.append(m)
    return maps


def kernel(input_current: np.ndarray, tau_mem: np.ndarray) -> np.ndarray:
    _ensure_concourse()
    from concourse.bass_utils import run_bass_kernel_spmd

    nc = _get_program()
    in_maps = prepare_in_maps(input_current, tau_mem, io="fp16")
    res = run_bass_kernel_spmd(nc, in_maps, list(range(N_CORES)))
    out = np.concatenate([res.results[c]["y"] for c in range(N_CORES)], axis=0)
    return out.astype(np.float32, copy=False)
